# revision 2
# baseline (speedup 1.0000x reference)
"""DeepSeekMoE v2: slot-packed expert-parallel on 8 trn2 NeuronCores.

Per-core SPMD program with 3 weight-set "slots" (column segments):
  slot0 (1024 cols = 512A+512B): core c<7 -> expert c main piece; core 7 -> shared
  slot1 ( 384 cols = 192A+192B): shared-expert tokens (flow-assigned)
  slot2 ( 256 cols = 128A+128B): core c<7 -> expert c overflow; core 7 -> shared
Every column runs gate/up/down; down output is scaled by its combine weight
(1.0 for shared columns) and scattered into part_a/part_b; two ReduceScatters
sum the 3 contributions (2 routed + 1 shared) of every token across cores.
out = concat(rs_a, rs_b) per core. No separate shared path, no post-RS add.

The shared-token assignment picks, for each token, a core that does NOT host
that token as a routed column (so per-core scatter rows stay unique).

Schedule: gate/up (22 i-chunks, streamed weights; part zeroing + IT/WT at
i==6, slot0 down-weights DW0 resident by i==19) -> slot1+slot2 down (k-outer,
d1/d2 streamed, 3+2 psum accumulators) + scatters -> slot0-A down (DW0) +
scatters -> RS_A (pipelines with producers) -> slot0-B down -> RS_B -> out
convert bf16->f32.
"""

import os
import sys
import types

import numpy as np
import ml_dtypes

for _p in ('/opt/trn_rl_repo', '/root/.axon_site/_ro/trn_rl_repo'):
    if os.path.isdir(_p) and _p not in sys.path:
        sys.path.append(_p)


def _install_profile_glue():
    try:
        import antenv
        if 'antenv.axon_hooks' in sys.modules:
            return
        mod = types.ModuleType('antenv.axon_hooks')
        holder = [None]
        mod.set_axon_ntff_profile_hook = lambda h: holder.__setitem__(0, h)
        mod.get_axon_ntff_profile_hook = lambda: holder[0]
        sys.modules['antenv.axon_hooks'] = mod
        antenv.axon_hooks = mod
        so = '/opt/axon/libaxon_pjrt.so'
        if os.path.exists(so):
            from trn_agent_boot.trn_boot import _ntff_profile_via_ctypes
            hook = _ntff_profile_via_ctypes(so)
            if hook is not None:
                mod.set_axon_ntff_profile_hook(hook)
    except Exception:
        pass


_install_profile_glue()

import concourse.bass as bass
import concourse.mybir as mybir
from concourse.bass_utils import run_bass_kernel_spmd
from concourse.tile import TileContext

B, S, H, I = 2, 2048, 1024, 2816
E_ROUTED = 7
TOP_K = 2
T = B * S
HALF = T // 2
NCORES = 8
KH = H // 128               # 8
KI = I // 128               # 22

F32 = mybir.dt.float32
BF16 = mybir.dt.bfloat16
I32 = mybir.dt.int32

PAD_IDX = 1 << 20

LAST_RESULT = None
_PROG_CACHE = {}

# slot capacity templates, tried in order until host assignment fits
TEMPLATES = [
    (512, 512, 192, 192, 128, 128),   # T=1664
    (512, 512, 256, 256, 192, 192),   # T=1920 fallback
    (640, 640, 320, 320, 256, 256),   # T=2432 paranoid fallback
]


def _split_sync_waits(nc, max_waits=1):
    for f in nc.m.functions:
        for bb in f.blocks:
            new_list = []
            changed = False
            for inst in bb.instructions:
                si = inst.sync_info
                if si is not None and si.on_wait is not None and len(si.on_wait) > max_waits:
                    waits = list(si.on_wait)
                    while len(waits) > max_waits:
                        chunk, waits = waits[:max_waits], waits[max_waits:]
                        nop = mybir.InstNoOp(
                            name=nc.get_next_instruction_name(),
                            engine=inst.engine, bass_nofuse=True,
                            sync_info=mybir.SyncInfo(on_wait=chunk, on_update=[]),
                        )
                        new_list.append(nop)
                    inst.sync_info = mybir.SyncInfo(
                        on_wait=waits, on_update=list(si.on_update or []))
                    changed = True
                new_list.append(inst)
            if changed:
                bb.instructions[:] = new_list


def _slot_layout(caps):
    """Returns (CT, slots, mtiles). slots: list of (start, capA, capB).
    mtiles: list of dicts {m, slot, c0, scatters:[(it_col, half, r0, rn)]}
    it_col counts primary columns 0..NM-1 then extra straddle columns."""
    s0a, s0b, s1a, s1b, s2a, s2b = caps
    slots = []
    start = 0
    for ca, cb in ((s0a, s0b), (s1a, s1b), (s2a, s2b)):
        assert (ca + cb) % 128 == 0 and ca % 64 == 0 and cb % 64 == 0
        slots.append((start, ca, cb))
        start += ca + cb
    CT = start
    NM = CT // 128
    mtiles = []
    extra_col = NM
    m = 0
    for si, (st, ca, cb) in enumerate(slots):
        for mt in range((ca + cb) // 128):
            c0 = st + mt * 128
            a_rows = max(0, min(128, st + ca - c0))   # rows in A region
            scat = []
            if a_rows > 0:
                scat.append((m, 0, 0, a_rows))        # (it_col, half, r0, rn)
            if a_rows < 128:
                if a_rows == 0:
                    scat.append((m, 1, 0, 128))
                else:
                    scat.append((extra_col, 1, a_rows, 128 - a_rows))
                    extra_col += 1
            mtiles.append(dict(m=m, slot=si, c0=c0, scatters=scat))
            m += 1
    return CT, NM, extra_col, slots, mtiles


def _build_program(caps, level=0):
    CT, NM, NIT, slots, mtiles = _slot_layout(caps)
    nc = bass.Bass()
    xt = nc.declare_dram_parameter('xt', [H, CT], BF16, isOutput=False)
    idx = nc.declare_dram_parameter('idx', [NIT * 128], I32, isOutput=False)
    wv = nc.declare_dram_parameter('wv', [NM * 128], F32, isOutput=False)
    gus = []
    for s in range(3):
        g = nc.declare_dram_parameter(f'g{s}', [KI, 128, KH, 128], BF16, isOutput=False)
        u = nc.declare_dram_parameter(f'u{s}', [KI, 128, KH, 128], BF16, isOutput=False)
        gus.append((g, u))
    d0 = nc.declare_dram_parameter('d0', [I, H], BF16, isOutput=False)
    d1 = nc.declare_dram_parameter('d1', [I, H], BF16, isOutput=False)
    d2 = nc.declare_dram_parameter('d2', [I, H], BF16, isOutput=False)
    out = nc.declare_dram_parameter('out', [T // NCORES, H], F32, isOutput=True)

    part = [nc.dram_tensor('part_a', [HALF, H], BF16),
            nc.dram_tensor('part_b', [HALF, H], BF16)]
    SHH = HALF // NCORES    # 256 rows per core per half
    rs = [nc.dram_tensor('rs_a', [SHH, H], BF16),
          nc.dram_tensor('rs_b', [SHH, H], BF16)]

    # gate/up column tiles (slot-aligned, <=512 wide)
    gu_tiles = []
    for si, (st, ca, cb) in enumerate(slots):
        w = ca + cb
        c = 0
        while c < w:
            cn = min(512, w - c)
            gu_tiles.append((si, st + c, cn))
            c += cn

    wbufs = 2 if level < 1 else 1
    with TileContext(nc) as tc:
        with (
            tc.tile_pool(name='big', bufs=1) as bigp,
            tc.tile_pool(name='wstream', bufs=wbufs) as wsp,
            tc.tile_pool(name='dstream', bufs=6) as dsp,
            tc.tile_pool(name='work', bufs=3 if level < 2 else 2) as wkp,
            tc.tile_pool(name='outp', bufs=2) as outp,
            tc.tile_pool(name='ps', bufs=4, space='PSUM') as psp,
        ):
            XT = bigp.tile([128, KH, CT], BF16, tag='XT')
            HT = bigp.tile([128, KI, CT], BF16, tag='HT')
            DW0 = bigp.tile([128, KI, H], BF16, tag='DW0')
            IT = bigp.tile([128, NIT], I32, tag='IT')
            WT = bigp.tile([128, NM], F32, tag='WT')
            ZT = bigp.tile([128, H], BF16, tag='ZT')
            xt_r = xt.rearrange('(k p) c -> p k c', p=128)

            nc.vector.memset(ZT[:, :], 0.0)

            # ---- phase 1: gate/up over all columns, i-chunk major
            for i in range(KI):
                if i == 0:
                    nc.sync.dma_start(out=XT[:, 0, :], in_=xt_r[:, 0, :])
                chs = []
                for s in range(3):
                    gch = wsp.tile([128, KH, 128], BF16, tag=f'g{s}', name=f'g{s}_{i}')
                    nc.sync.dma_start(out=gch[:, :, :], in_=gus[s][0][i, :, :, :])
                    uch = wsp.tile([128, KH, 128], BF16, tag=f'u{s}', name=f'u{s}_{i}')
                    nc.sync.dma_start(out=uch[:, :, :], in_=gus[s][1][i, :, :, :])
                    chs.append((gch, uch))
                if i == 0:
                    for k in range(1, KH):
                        nc.sync.dma_start(out=XT[:, k, :], in_=xt_r[:, k, :])
                for (si, c0, cn) in gu_tiles:
                    gch, uch = chs[si]
                    ps = psp.tile([128, 1024], F32, tag='ps', name=f'p{i}_{c0}')
                    for k in range(KH):
                        nc.tensor.matmul(
                            ps[:, :cn], lhsT=gch[:, k, :], rhs=XT[:, k, c0:c0 + cn],
                            start=(k == 0), stop=(k == KH - 1))
                    for k in range(KH):
                        nc.tensor.matmul(
                            ps[:, 512:512 + cn], lhsT=uch[:, k, :],
                            rhs=XT[:, k, c0:c0 + cn],
                            start=(k == 0), stop=(k == KH - 1), skip_group_check=True)
                    at = wkp.tile([128, 512], F32, tag='at', name=f'at{i}_{c0}')
                    nc.scalar.activation(
                        out=at[:, :cn], in_=ps[:, :cn],
                        func=mybir.ActivationFunctionType.Silu)
                    nc.vector.tensor_tensor(
                        out=HT[:, i, c0:c0 + cn],
                        in0=at[:, :cn], in1=ps[:, 512:512 + cn],
                        op=mybir.AluOpType.mult)
                if i == 6:
                    nc.gpsimd.dma_start(out=IT[:, :], in_=idx.rearrange('(m p) -> p m', p=128))
                    nc.gpsimd.dma_start(out=WT[:, :], in_=wv.rearrange('(m p) -> p m', p=128))
                    for h in range(2):
                        for r in range(HALF // 128):
                            nc.gpsimd.dma_start(
                                out=part[h][r * 128:(r + 1) * 128, :], in_=ZT[:, :])
                if i == KI - 3:
                    for k in range(KI):
                        nc.sync.dma_start(
                            out=DW0[:, k, :], in_=d0[k * 128:(k + 1) * 128, :])

            def scale_and_scatter(mt, pm):
                m = mt['m']
                ysb = wkp.tile([128, H], BF16, tag='ysb', name=f'ysb{m}')
                nc.vector.tensor_scalar_mul(ysb[:, :], pm[:, :], WT[:, m:m + 1])
                for (it_col, half, r0, rn) in mt['scatters']:
                    nc.gpsimd.indirect_dma_start(
                        out=part[half][:, :],
                        out_offset=bass.IndirectOffsetOnAxis(
                            ap=IT[:, it_col:it_col + 1], axis=0),
                        in_=ysb[:, :], in_offset=None,
                        bounds_check=HALF - 1, oob_is_err=False)

            # ---- phase 2: slot1 + slot2 down, k-outer with streamed d1/d2
            for si, dpar in ((1, d1), (2, d2)):
                mts = [mt for mt in mtiles if mt['slot'] == si]
                pms = [psp.tile([128, H], F32, tag='ps', name=f'pd{si}_{mt["m"]}')
                       for mt in mts]
                for k in range(KI):
                    dch = dsp.tile([128, H], BF16, tag='dstr', name=f'd{si}_{k}')
                    nc.sync.dma_start(out=dch[:, :], in_=dpar[k * 128:(k + 1) * 128, :])
                    for mt, pm in zip(mts, pms):
                        c0 = mt['c0']
                        for n in range(2):
                            nc.tensor.matmul(
                                pm[:, n * 512:(n + 1) * 512],
                                lhsT=HT[:, k, c0:c0 + 128],
                                rhs=dch[:, n * 512:(n + 1) * 512],
                                start=(k == 0), stop=(k == KI - 1),
                                skip_group_check=True)
                for mt, pm in zip(mts, pms):
                    scale_and_scatter(mt, pm)

            # ---- phases 3/4: slot0 down per half (DW0 resident), RS per half
            s0_mts = [mt for mt in mtiles if mt['slot'] == 0]
            nA = sum(1 for mt in s0_mts
                     if any(h == 0 for (_, h, _, _) in mt['scatters']))
            for half in range(2):
                group = s0_mts[:nA] if half == 0 else s0_mts[nA:]
                for mt in group:
                    c0 = mt['c0']
                    pm = psp.tile([128, H], F32, tag='ps', name=f'pd0_{mt["m"]}')
                    for k in range(KI):
                        for n in range(2):
                            nc.tensor.matmul(
                                pm[:, n * 512:(n + 1) * 512],
                                lhsT=HT[:, k, c0:c0 + 128],
                                rhs=DW0[:, k, n * 512:(n + 1) * 512],
                                start=(k == 0), stop=(k == KI - 1),
                                skip_group_check=True)
                    scale_and_scatter(mt, pm)
                nc.gpsimd.collective_compute(
                    'ReduceScatter', mybir.AluOpType.add,
                    replica_groups=[list(range(NCORES))],
                    ins=[part[half][:, :]], outs=[rs[half][:, :]])

            # ---- phase 5: out = concat(rs_a, rs_b) converted to f32
            for half in range(2):
                for j in range(SHH // 128):
                    rt = outp.tile([128, H], BF16, tag='rt', name=f'rt{half}_{j}')
                    nc.sync.dma_start(
                        out=rt[:, :], in_=rs[half][j * 128:(j + 1) * 128, :])
                    cf = outp.tile([128, H], F32, tag='cf', name=f'cf{half}_{j}')
                    # scalar engine: keeps the strict-FIFO DVE queue free of
                    # RS-gated work (head-of-line blocking of phase-4 scales)
                    nc.scalar.copy(cf[:, :], rt[:, :])
                    ro = half * SHH + j * 128
                    nc.sync.dma_start(out=out[ro:ro + 128, :], in_=cf[:, :])

    _split_sync_waits(nc)
    return nc


# ---------------- host side ----------------

def _shuffle_gateup(wmat):
    return np.ascontiguousarray(
        wmat.reshape(KH, 128, KI, 128).transpose(2, 1, 0, 3).astype(ml_dtypes.bfloat16))


def _dispatch(x2, router_w, routing_bias):
    logits = x2 @ router_w + routing_bias
    order = np.argsort(-logits, axis=1, kind='stable')[:, :TOP_K]
    probs = 1.0 / (1.0 + np.exp(-logits))
    rows = np.arange(T)
    s = probs[rows[:, None], order]
    w = s / s.sum(axis=1, keepdims=True)
    return order, w


def _assign(order, w, caps):
    """Returns per-core slot contents or None if this template doesn't fit.
    cores[c]['slots'][si] = (listA, listB) of (token, weight)."""
    s0a, s0b, s1a, s1b, s2a, s2b = caps
    ex = [[[], []] for _ in range(E_ROUTED)]
    for t in range(T):
        for k in range(TOP_K):
            e = order[t, k]
            ex[e][0 if t < HALF else 1].append((t, float(w[t, k])))
    cores = []
    for c in range(E_ROUTED):
        la, lb = ex[c]
        if len(la) > s0a + s2a or len(lb) > s0b + s2b:
            return None
        cores.append({'slots': [(la[:s0a], lb[:s0b]), None,
                                (la[s0a:], lb[s0b:])]})
    cores.append({'slots': [None, None, None]})   # core 7: all shared

    # shared-token assignment per half: token t may go to any core not hosting
    # t as a routed column (cores 0-6 host exactly their expert's tokens);
    # core 7 hosts nothing routed and acts as the big sink.
    hosts = [set(int(e) for e in order[t]) for t in range(T)]
    shared = [[[] for _ in range(NCORES)] for _ in range(2)]
    for half in range(2):
        qa = [s1a] * E_ROUTED + [s0a + s1a + s2a] if half == 0 else \
             [s1b] * E_ROUTED + [s0b + s1b + s2b]
        rem = list(qa)
        for t in range(half * HALF, (half + 1) * HALF):
            best, bq = 7, -1
            for c in range(E_ROUTED):
                if c not in hosts[t] and rem[c] > bq and rem[c] > 0:
                    best, bq = c, rem[c]
            if best == 7 and rem[7] <= 0:
                return None
            shared[half][best].append((t, 1.0))
            rem[best] -= 1
    for c in range(E_ROUTED):
        a, b = shared[0][c], shared[1][c]
        if len(a) > s1a or len(b) > s1b:
            return None
        cores[c]['slots'][1] = (a, b)
    a, b = shared[0][7], shared[1][7]
    if len(a) > s0a + s1a + s2a or len(b) > s0b + s1b + s2b:
        return None
    cores[7]['slots'][0] = (a[:s0a], b[:s0b])
    cores[7]['slots'][1] = (a[s0a:s0a + s1a], b[s0b:s0b + s1b])
    cores[7]['slots'][2] = (a[s0a + s1a:], b[s0b + s1b:])
    return cores


def kernel(x, router_w, routing_bias, shared_gate, shared_up, shared_down,
           routed_gate, routed_up, routed_down):
    global LAST_RESULT
    x = np.asarray(x, np.float32)
    x2 = x.reshape(T, H)
    order, w = _dispatch(x2, np.asarray(router_w, np.float32),
                         np.asarray(routing_bias, np.float32))

    caps = cores = None
    for caps_try in TEMPLATES:
        cores = _assign(order, w, caps_try)
        if cores is not None:
            caps = caps_try
            break
    assert cores is not None, 'no slot template fits this routing'

    CT, NM, NIT, slots, mtiles = _slot_layout(caps)

    bf = ml_dtypes.bfloat16
    routed_gate = np.asarray(routed_gate, np.float32)
    routed_up = np.asarray(routed_up, np.float32)
    routed_down = np.asarray(routed_down, np.float32)
    gw_s = [_shuffle_gateup(routed_gate[e]) for e in range(E_ROUTED)]
    uw_s = [_shuffle_gateup(routed_up[e]) for e in range(E_ROUTED)]
    dw_b = [np.ascontiguousarray(routed_down[e].astype(bf)) for e in range(E_ROUTED)]
    sgw_s = _shuffle_gateup(np.asarray(shared_gate, np.float32))
    suw_s = _shuffle_gateup(np.asarray(shared_up, np.float32))
    sdw_b = np.ascontiguousarray(np.asarray(shared_down, np.float32).astype(bf))

    in_maps = []
    for c in range(NCORES):
        sl = cores[c]['slots']
        xt_h = np.zeros((CT, H), np.float32)
        idx_h = np.full((NIT * 128,), PAD_IDX, np.int32)
        wv_h = np.zeros((NM * 128,), np.float32)   # column-major == m-tile-major
        colmap = np.full((CT,), -1, np.int64)
        for si, (st, ca, cb) in enumerate(slots):
            la, lb_ = sl[si]
            for off, capn, toks in ((st, ca, la), (st + ca, cb, lb_)):
                assert len(toks) <= capn
                if toks:
                    tok_ids = np.array([t for t, _ in toks], np.int64)
                    xt_h[off:off + len(toks)] = x2[tok_ids]
                    colmap[off:off + len(toks)] = tok_ids
                    wv_h[off:off + len(toks)] = [wt for _, wt in toks]
        for mt in mtiles:
            for (it_col, half, r0, rn) in mt['scatters']:
                for r in range(r0, r0 + rn):
                    t = colmap[mt['c0'] + r]
                    if t >= 0:
                        idx_h[it_col * 128 + r] = t - half * HALF

        if c < E_ROUTED:
            g0, u0, d0_ = gw_s[c], uw_s[c], dw_b[c]
            g2, u2, d2_ = gw_s[c], uw_s[c], dw_b[c]
        else:
            g0, u0, d0_ = sgw_s, suw_s, sdw_b
            g2, u2, d2_ = sgw_s, suw_s, sdw_b
        in_maps.append({
            'xt': np.ascontiguousarray(xt_h.T.astype(bf)),
            'idx': idx_h, 'wv': wv_h,
            'g0': g0, 'u0': u0, 'd0': d0_,
            'g1': sgw_s, 'u1': suw_s, 'd1': sdw_b,
            'g2': g2, 'u2': u2, 'd2': d2_,
        })

    key = caps
    nc = _PROG_CACHE.get(key)
    if nc is None:
        last_err = None
        for level in range(3):
            try:
                nc = _build_program(key, level)
                break
            except ValueError as e:
                last_err = e
        else:
            raise last_err
        _PROG_CACHE[key] = nc

    res = run_bass_kernel_spmd(nc, in_maps, list(range(NCORES)))
    LAST_RESULT = res

    SHH = HALF // NCORES
    out_full = np.empty((T, H), np.float32)
    for c in range(NCORES):
        sset = np.concatenate([
            np.arange(c * SHH, (c + 1) * SHH),
            HALF + np.arange(c * SHH, (c + 1) * SHH)])
        out_full[sset] = res.results[c]['out']
    return out_full.reshape(B, S, H)
